# revision 9
# baseline (speedup 1.0000x reference)
"""Trainium2 Bass kernel for nn_Attention_88613765251714.

Single-head causal attention with RoPE, B=4 S=2048 D=2048 fp32.

Sharding: 8 cores = 4 batches x 2 cores/batch. Within a batch pair:
 - core parity h owns sequence half h for the K and W' projections
   (exchanged pairwise via AllGather),
 - query blocks (16 x 128 rows) are split between the pair in a
   load-balanced interleaving; each core computes Q projection, attention
   and output for its own 1024 query rows.

Key algebraic fold: out = softmax(S) @ V @ Wo.T + bo
                        = softmax(S) @ (x @ (Wo @ Wv).T + (Wo bv + bo))
so the V and output projections collapse into ONE projection with the
host-precomputed Wf = Wo @ Wv; the per-row bias (Wo bv + bo) rides along
because softmax rows sum to 1. This removes the entire output-projection
matmul stage (~21% of tensor-engine rows).

Attention blocks are software-pipelined: the scores matmuls of block j+1
are emitted before the transpose + P@W' of block j, so the tensor engine
keeps streaming while the vector/activation engines run block j's softmax.
Slots are ordered small-to-large so attention starts before K/W' tiles
finish loading.

On-device compute uses fp16 matmul operands (fp32 PSUM accumulation),
fp32 softmax. The causal structure is exploited by giving each query-block
"slot" a fixed key extent (structure shared by all cores so one SPMD
program serves all 8); the actual mask enters as an additive bias built
on the host, so non-causal masks fall back to full-extent slots.
"""
import sys
sys.path.insert(0, '/opt/trn_rl_repo')
import math
from contextlib import ExitStack

import numpy as np

import concourse.bass as bass  # noqa: F401  (registers engines)
import concourse.mybir as mybir
import concourse.tile as tile
from concourse import bacc
from concourse.masks import make_identity

F32 = mybir.dt.float32
F16 = mybir.dt.float16

N_CORES = 8
B, S, D = 4, 2048, 2048
P = 128
NBLK = S // P            # 16 query blocks per batch
SQ = S // 2              # 1024 query rows per core
DCH = D // P             # 16 feature chunks
HALF = D // 2            # rope half dim

CAUSAL_SLOT_CHUNKS = [2, 4, 6, 8, 10, 12, 14, 16]
BLOCKS_EVEN = [0, 2, 4, 6, 9, 11, 13, 15]
BLOCKS_ODD = [1, 3, 5, 7, 8, 10, 12, 14]
FULL_SLOT_CHUNKS = [16] * 8

REPLICA_GROUPS = [[0, 1], [2, 3], [4, 5], [6, 7]]
NEG = -30000.0


def _proj_to_eT(nc, tc, ctx, w_dram, x_sb, bias_sb, out_raw, psum_pool, wpool):
    """out_raw[e, s] = (x @ W.T + b).T for x given as xT in SBUF.

    w_dram: [D, E] (= W.T, host-transposed), x_sb: [128, DCH, SQ] f16,
    bias_sb: [128, ECH] f32, out_raw: [128, ECH, SQ] f32 SBUF.
    """
    ech = out_raw.shape[1]
    sgroups = x_sb.shape[2] // 512
    for e2 in range(ech // 2):
        wts = []
        for d in range(DCH):
            wt = wpool.tile([P, 256], F16, tag="w")
            nc.sync.dma_start(wt[:], w_dram[e2, d])
            wts.append(wt)
        for es in range(2):
            e = e2 * 2 + es
            for sg in range(sgroups):
                ps = psum_pool.tile([P, 512], F32, tag="mm512")
                for d in range(DCH):
                    nc.tensor.matmul(
                        ps[:], wts[d][:, es * P:(es + 1) * P],
                        x_sb[:, d, sg * 512:(sg + 1) * 512],
                        start=(d == 0), stop=(d == DCH - 1))
                nc.vector.tensor_scalar_add(
                    out_raw[:, e, sg * 512:(sg + 1) * 512], ps[:], bias_sb[:, e:e + 1])


def _rope_to_stage(nc, raw, cos_sb, sin_sb, stage_dram, tmp_pool):
    """raw: [128, DCH, SQ] f32 (feature-major), cos/sin: [128, HALF//P, SQ] f32.
    Writes rope(raw) as f16 to stage_dram [D, SQ]."""
    hch = HALF // P  # 8
    sq = raw.shape[2]
    for c in range(hch):
        t1 = tmp_pool.tile([P, sq], F32, tag="rt1")
        t2 = tmp_pool.tile([P, sq], F32, tag="rt2")
        lo = tmp_pool.tile([P, sq], F16, tag="rlo")
        nc.vector.tensor_mul(t1[:], raw[:, c], cos_sb[:, c])
        nc.vector.tensor_mul(t2[:], raw[:, c + hch], sin_sb[:, c])
        nc.vector.tensor_sub(lo[:], t1[:], t2[:])
        nc.sync.dma_start(stage_dram[c * P:(c + 1) * P, :], lo[:])
        t3 = tmp_pool.tile([P, sq], F32, tag="rt1")
        t4 = tmp_pool.tile([P, sq], F32, tag="rt2")
        hi = tmp_pool.tile([P, sq], F16, tag="rlo")
        nc.vector.tensor_mul(t3[:], raw[:, c], sin_sb[:, c])
        nc.vector.tensor_mul(t4[:], raw[:, c + hch], cos_sb[:, c])
        nc.vector.tensor_add(hi[:], t3[:], t4[:])
        nc.sync.dma_start(stage_dram[(c + hch) * P:(c + hch + 1) * P, :], hi[:])


def build_program(slot_chunks, repeat=1, phases="all"):
    slot_chunks = list(slot_chunks)
    total_cols = sum(slot_chunks) * P
    nc = bacc.Bacc("TRN2", target_bir_lowering=False, debug=False, num_devices=N_CORES)

    xq_t = nc.dram_tensor("xq_t", [D, SQ], F16, kind="ExternalInput")
    xkv_t = nc.dram_tensor("xkv_t", [D, SQ], F16, kind="ExternalInput")
    wq_t = nc.dram_tensor("wq_tl", [D // 256, DCH, P, 256], F16, kind="ExternalInput")
    wk_t = nc.dram_tensor("wk_tl", [D // 256, DCH, P, 256], F16, kind="ExternalInput")
    wf_t = nc.dram_tensor("wf_t", [D, D], F16, kind="ExternalInput")
    bq_d = nc.dram_tensor("bq", [D], F32, kind="ExternalInput")
    bk_d = nc.dram_tensor("bk", [D], F32, kind="ExternalInput")
    bfo_d = nc.dram_tensor("bfo16", [D], F16, kind="ExternalInput")
    cosq_d = nc.dram_tensor("cosq", [HALF, SQ], F16, kind="ExternalInput")
    sinq_d = nc.dram_tensor("sinq", [HALF, SQ], F16, kind="ExternalInput")
    cosk_d = nc.dram_tensor("cosk", [HALF, SQ], F16, kind="ExternalInput")
    sink_d = nc.dram_tensor("sink", [HALF, SQ], F16, kind="ExternalInput")
    mbias_d = nc.dram_tensor("mbias", [P, total_cols], F16, kind="ExternalInput")
    out_d = nc.dram_tensor("out", [SQ, D], F32, kind="ExternalOutput")

    with tile.TileContext(nc) as tc, ExitStack() as ctx:
        dram = ctx.enter_context(tc.tile_pool(name="dram", bufs=1, space="DRAM"))
        const = ctx.enter_context(tc.tile_pool(name="const", bufs=1))

        ident = const.tile([P, P], F16)
        make_identity(nc, ident[:])
        ones1 = const.tile([1, P], F16)
        nc.vector.memset(ones1[:], 1.0)
        bq_sb = const.tile([P, DCH], F32)
        nc.sync.dma_start(bq_sb[:], bq_d.ap().rearrange("(o p) -> p o", p=P))
        bk_sb = const.tile([P, DCH], F32)
        nc.sync.dma_start(bk_sb[:], bk_d.ap().rearrange("(o p) -> p o", p=P))

        for _rep in range(repeat):
          kstage = dram.tile([D, SQ], F16)
          wstage_a = dram.tile([SQ // 2, D], F16)
          wstage_b = dram.tile([SQ // 2, D], F16)
          qstage = dram.tile([D, SQ], F16)
          kgather = dram.tile([2, D, SQ], F16)
          wgather_a = dram.tile([2, SQ // 2, D], F16)
          wgather_b = dram.tile([2, SQ // 2, D], F16)
          if _rep == repeat - 1:
              out_ap = out_d.ap()
          else:
              out_scratch = dram.tile([SQ, D], F32, name=f"out_scratch_{_rep}")
              out_ap = out_scratch[:]
          if phases == "none":
              ot = const.tile([1, 512], F32, name=f"dummy_out0_{_rep}")
              nc.vector.memset(ot[:], 1.0)
              nc.sync.dma_start(out_ap[0:1, 0:512], ot[:])
              continue

          # ---------------- P1: projections + allgather ----------------
          with tc.tile_pool(name="psumP", bufs=6, space="PSUM") as psum_pool:
            with tc.tile_pool(name="kvx", bufs=1) as kvx:
              bfo_row = kvx.tile([1, D], F16, name=f"bfo_row_{_rep}")
              nc.sync.dma_start(bfo_row[:], bfo_d.ap()[None, :])
              xkv_sb = kvx.tile([P, DCH, SQ], F16)
              xkv_r = xkv_t.ap().rearrange("(do di) s -> di do s", di=P)
              for d in range(DCH):
                  nc.sync.dma_start(xkv_sb[:, d], xkv_r[:, d])

              # K projection + rope -> kstage -> allgather
              with tc.tile_pool(name="kp", bufs=1) as kp, \
                   tc.tile_pool(name="kw", bufs=32) as kw, \
                   tc.tile_pool(name="kt", bufs=1) as ktmp:
                  cosk_sb = kp.tile([P, HALF // P, SQ], F16)
                  nc.sync.dma_start(cosk_sb[:], cosk_d.ap().rearrange("(ho hi) s -> hi ho s", hi=P))
                  sink_sb = kp.tile([P, HALF // P, SQ], F16)
                  nc.sync.dma_start(sink_sb[:], sink_d.ap().rearrange("(ho hi) s -> hi ho s", hi=P))
                  kraw = kp.tile([P, DCH, SQ], F32)
                  _proj_to_eT(nc, tc, ctx, wk_t.ap(), xkv_sb, bk_sb, kraw, psum_pool, kw)
                  _rope_to_stage(nc, kraw, cosk_sb, sink_sb, kstage, ktmp)
              nc.gpsimd.collective_compute(
                  "AllGather", mybir.AluOpType.bypass, replica_groups=REPLICA_GROUPS,
                  ins=[kstage[:]], outs=[kgather[:]])

              # W' = x @ Wf.T + (Wo bv + bo) projection -> wstage -> allgather
              with tc.tile_pool(name="vw", bufs=1) as vw, \
                   tc.tile_pool(name="vs", bufs=3) as vstg:
                  wf_tiles = []
                  for d in range(DCH):
                      wt = vw.tile([P, D], F16, name=f"wf_{d}")
                      nc.sync.dma_start(wt[:], wf_t.ap()[d * P:(d + 1) * P, :])
                      wf_tiles.append(wt)
                  for sc in range(SQ // P):
                      whalf, wrow = (wstage_a, sc) if sc < 4 else (wstage_b, sc - 4)
                      for eg in range(4):
                          ps = psum_pool.tile([P, 512], F32, tag="mm512")
                          for d in range(DCH):
                              nc.tensor.matmul(
                                  ps[:], xkv_sb[:, d, sc * P:(sc + 1) * P],
                                  wf_tiles[d][:, eg * 512:(eg + 1) * 512],
                                  start=(d == 0), stop=False)
                          nc.tensor.matmul(
                              ps[:], ones1[:], bfo_row[:, eg * 512:(eg + 1) * 512],
                              start=False, stop=True)
                          st = vstg.tile([P, 512], F16, tag="vst")
                          nc.vector.tensor_copy(st[:], ps[:])
                          nc.sync.dma_start(
                              whalf[wrow * P:(wrow + 1) * P, eg * 512:(eg + 1) * 512], st[:])
                      if sc == 3:
                          nc.gpsimd.collective_compute(
                              "AllGather", mybir.AluOpType.bypass,
                              replica_groups=REPLICA_GROUPS,
                              ins=[wstage_a[:]], outs=[wgather_a[:]])
              nc.gpsimd.collective_compute(
                  "AllGather", mybir.AluOpType.bypass, replica_groups=REPLICA_GROUPS,
                  ins=[wstage_b[:]], outs=[wgather_b[:]])

            # Q projection + rope -> qstage (overlaps the allgathers)
            with tc.tile_pool(name="qp", bufs=1) as qp, \
                 tc.tile_pool(name="qw", bufs=32) as qw, \
                 tc.tile_pool(name="qt", bufs=1) as qtmp:
                xq_sb = qp.tile([P, DCH, SQ], F16)
                xq_r = xq_t.ap().rearrange("(do di) s -> di do s", di=P)
                for d in range(DCH):
                    nc.sync.dma_start(xq_sb[:, d], xq_r[:, d])
                cosq_sb = qp.tile([P, HALF // P, SQ], F16)
                nc.sync.dma_start(cosq_sb[:], cosq_d.ap().rearrange("(ho hi) s -> hi ho s", hi=P))
                sinq_sb = qp.tile([P, HALF // P, SQ], F16)
                nc.sync.dma_start(sinq_sb[:], sinq_d.ap().rearrange("(ho hi) s -> hi ho s", hi=P))
                qraw = qp.tile([P, DCH, SQ], F32)
                _proj_to_eT(nc, tc, ctx, wq_t.ap(), xq_sb, bq_sb, qraw, psum_pool, qw)
                _rope_to_stage(nc, qraw, cosq_sb, sinq_sb, qstage, qtmp)

          if phases == "p1":
              ot = const.tile([1, 512], F32, name=f"dummy_out_{_rep}")
              nc.vector.memset(ot[:], 1.0)
              nc.sync.dma_start(out_ap[0:1, 0:512], ot[:])
              continue

          # -------- P2: attention (+ folded output), block-pipelined --------
          with tc.tile_pool(name="attr", bufs=1) as attr, \
               tc.tile_pool(name="slot2", bufs=2) as sl2, \
               tc.tile_pool(name="slots", bufs=2) as sls, \
               tc.tile_pool(name="slot1", bufs=2) as sl1, \
               tc.tile_pool(name="ost", bufs=3) as ost, \
               tc.tile_pool(name="psA", bufs=4, space="PSUM") as psA, \
               tc.tile_pool(name="psB", bufs=2, space="PSUM") as psB, \
               tc.tile_pool(name="psT", bufs=2, space="PSUM") as psT:
              kT_sb = attr.tile([P, DCH, S], F16)
              for half in range(2):
                  src = kgather[half].rearrange("(do di) s -> di do s", di=P)
                  for sub in range(2):
                      nc.scalar.dma_start(
                          kT_sb[:, :, half * SQ + sub * 512:half * SQ + (sub + 1) * 512],
                          src[:, :, sub * 512:(sub + 1) * 512])
              w_sb = attr.tile([P, NBLK, D], F16)
              nc.gpsimd.dma_start(
                  w_sb[:, 0:4, :],
                  wgather_a[0].rearrange("(co ci) e -> ci co e", ci=P))
              nc.gpsimd.dma_start(
                  w_sb[:, 4:8, :],
                  wgather_b[0].rearrange("(co ci) e -> ci co e", ci=P))
              nc.gpsimd.dma_start(
                  w_sb[:, 8:12, :],
                  wgather_a[1].rearrange("(co ci) e -> ci co e", ci=P))
              nc.gpsimd.dma_start(
                  w_sb[:, 12:16, :],
                  wgather_b[1].rearrange("(co ci) e -> ci co e", ci=P))

              def emit_tail(st):
                  kc_, j_, pexp_, linv_ = st
                  pT = sl1.tile([P, NBLK, P], F16, tag="pT")
                  for c in range(kc_):
                      pst = psT.tile([P, P], F16, tag="pst")
                      nc.tensor.transpose(pst[:], pexp_[:, c * P:(c + 1) * P], ident[:])
                      nc.vector.tensor_copy(pT[:, c, :], pst[:])
                  for eg in range(4):
                      pc = psB.tile([P, 512], F32, tag="pw")
                      for c in range(kc_):
                          nc.tensor.matmul(
                              pc[:], pT[:, c, :], w_sb[:, c, eg * 512:(eg + 1) * 512],
                              start=(c == 0), stop=(c == kc_ - 1))
                      ot = ost.tile([P, 512], F32, tag="ot")
                      nc.vector.tensor_scalar_mul(ot[:], pc[:], linv_[:])
                      nc.sync.dma_start(
                          out_ap[j_ * P:(j_ + 1) * P, eg * 512:(eg + 1) * 512], ot[:])

              off = 0
              prev = None
              for j, kc in enumerate(slot_chunks):
                  kw_cols = kc * P
                  qt = sl2.tile([P, DCH, P], F16, tag="qt")
                  nc.sync.dma_start(
                      qt[:], qstage[:, j * P:(j + 1) * P]
                      .rearrange("(do di) s -> di do s", di=P))
                  mb = sl2.tile([P, 2048], F16, tag="mb")
                  nc.sync.dma_start(mb[:, :kw_cols], mbias_d.ap()[:, off:off + kw_cols])
                  s_sb = sl1.tile([P, 2048], F32, tag="s")
                  for kg in range((kc + 3) // 4):
                      width = min(512, kw_cols - kg * 512)
                      ps = psA.tile([P, 512], F32, tag="sc")
                      for d in range(DCH):
                          nc.tensor.matmul(
                              ps[:, :width], qt[:, d, :],
                              kT_sb[:, d, kg * 512:kg * 512 + width],
                              start=(d == 0), stop=(d == DCH - 1))
                      nc.vector.tensor_add(
                          s_sb[:, kg * 512:kg * 512 + width],
                          ps[:, :width], mb[:, kg * 512:kg * 512 + width])
                  nm = sls.tile([P, 1], F32, tag="nm")
                  nc.vector.reduce_max(
                      nm[:], s_sb[:, :kw_cols], axis=mybir.AxisListType.X, negate=True)
                  lsum = sls.tile([P, 1], F32, tag="lsum")
                  pexp = sl1.tile([P, 2048], F16, tag="pexp")
                  nc.scalar.activation(
                      pexp[:, :kw_cols], s_sb[:, :kw_cols],
                      mybir.ActivationFunctionType.Exp,
                      bias=nm[:], accum_out=lsum[:])
                  linv = sls.tile([P, 1], F32, tag="linv")
                  nc.vector.reciprocal(linv[:], lsum[:])
                  if prev is not None:
                      emit_tail(prev)
                  prev = (kc, j, pexp, linv)
                  off += kw_cols
              emit_tail(prev)

    nc.compile()
    return nc


# ---------------- host side ----------------

_CACHE = {}


def _get_runner(slot_key):
    if slot_key not in _CACHE:
        nc = build_program(list(slot_key))
        from concourse.bass_utils import run_bass_kernel_spmd  # noqa: F401
        _CACHE[slot_key] = nc
    return _CACHE[slot_key]


def _tile_w(W):
    wt = np.ascontiguousarray(W.T).astype(np.float16)          # [D, E]
    wt = wt.reshape(DCH, P, D // 256, 256)                     # [d_out, d_in, e2, 256]
    return np.ascontiguousarray(wt.transpose(2, 0, 1, 3))      # [e2, d_out, 128, 256]


def _host_inputs(x, mask, Wq, bq, Wk, bk, Wv, bv, Wo, bo, slot_chunks, causal):
    """Build the 8 per-core input dicts."""
    scale = 1.0 / math.sqrt(D)
    inv_freq = 1.0 / (10000.0 ** (np.arange(HALF, dtype=np.float64) / HALF))
    pos = np.arange(S, dtype=np.float64)
    ang = pos[:, None] * inv_freq[None, :]          # [S, HALF]
    cos_full = np.cos(ang).astype(np.float32)       # [S, HALF]
    sin_full = np.sin(ang).astype(np.float32)

    Wf = np.asarray(Wo, np.float32) @ np.asarray(Wv, np.float32)
    bfo = np.asarray(Wo, np.float32) @ np.asarray(bv, np.float32) \
        + np.asarray(bo, np.float32)

    shared = {
        "wq_tl": _tile_w(Wq),
        "wk_tl": _tile_w(Wk),
        "wf_t": np.ascontiguousarray(Wf.T).astype(np.float16),
        "bq": np.asarray(bq, np.float32), "bk": np.asarray(bk, np.float32),
        "bfo16": bfo.astype(np.float16),
    }

    in_maps = []
    meta = []
    for c in range(N_CORES):
        b, h = c // 2, c % 2
        blocks = (BLOCKS_EVEN if h == 0 else BLOCKS_ODD)
        qrows = np.concatenate([np.arange(blk * P, (blk + 1) * P) for blk in blocks])
        kvrows = np.arange(h * SQ, (h + 1) * SQ)
        m = dict(shared)
        m["xq_t"] = np.ascontiguousarray(x[b][qrows].T).astype(np.float16)
        m["xkv_t"] = np.ascontiguousarray(x[b][kvrows].T).astype(np.float16)
        m["cosq"] = np.ascontiguousarray(cos_full[qrows].T * scale).astype(np.float16)
        m["sinq"] = np.ascontiguousarray(sin_full[qrows].T * scale).astype(np.float16)
        m["cosk"] = np.ascontiguousarray(cos_full[kvrows].T).astype(np.float16)
        m["sink"] = np.ascontiguousarray(sin_full[kvrows].T).astype(np.float16)
        mb_parts = []
        for j, kc in enumerate(slot_chunks):
            blk = blocks[j]
            rows = slice(blk * P, (blk + 1) * P)
            mm = mask[b, rows, :kc * P]
            mb_parts.append(np.where(mm == 0, np.float16(NEG), np.float16(0.0)))
        m["mbias"] = np.concatenate(mb_parts, axis=1).astype(np.float16)
        in_maps.append(m)
        meta.append((b, blocks))
    return in_maps, meta


def kernel(**inputs):
    x = np.asarray(inputs["x"], np.float32)
    mask = np.asarray(inputs["mask"])
    args = {k: np.asarray(inputs[k]) for k in
            ["Wq", "bq", "Wk", "bk", "Wv", "bv", "Wo", "bo"]}

    tril = np.tril(np.ones((S, S), dtype=mask.dtype))
    causal = all(np.array_equal(mask[b], tril) for b in range(B))
    slot_chunks = CAUSAL_SLOT_CHUNKS if causal else FULL_SLOT_CHUNKS

    in_maps, meta = _host_inputs(
        x, mask, args["Wq"], args["bq"], args["Wk"], args["bk"],
        args["Wv"], args["bv"], args["Wo"], args["bo"], slot_chunks, causal)

    nc = _get_runner(tuple(slot_chunks))
    from concourse.bass_utils import run_bass_kernel_spmd
    res = run_bass_kernel_spmd(nc, in_maps, list(range(N_CORES)))

    out = np.empty((B, S, D), np.float32)
    for c in range(N_CORES):
        b, blocks = meta[c]
        oc = res.results[c]["out"]
        for j, blk in enumerate(blocks):
            out[b, blk * P:(blk + 1) * P, :] = oc[j * P:(j + 1) * P, :]
    return out


# revision 17
# speedup vs baseline: 1.3688x; 1.3688x over previous
"""Trainium2 Bass kernel for nn_Attention_88613765251714.

Single-head causal attention with RoPE, B=4 S=2048 D=2048 fp32.

Sharding: 8 cores = 4 batches x 2 cores/batch. Within a batch pair:
 - core parity h owns sequence half h for the K and W' projections
   (exchanged pairwise via AllGather),
 - query blocks (16 x 128 rows) are split between the pair in a
   load-balanced interleaving; each core computes Q projection, attention
   and output for its own 1024 query rows.

Key algebraic fold: out = softmax(S) @ V @ Wo.T + bo
                        = softmax(S) @ (x @ (Wo @ Wv).T + (Wo bv + bo))
so the V and output projections collapse into ONE projection with the
host-precomputed Wf = Wo @ Wv; the per-row bias (Wo bv + bo) rides along
because softmax rows sum to 1. This removes the entire output-projection
matmul stage (~21% of tensor-engine rows).

Attention blocks are software-pipelined: the scores matmuls of block j+1
are emitted before the transpose + P@W' of block j, so the tensor engine
keeps streaming while the vector/activation engines run block j's softmax.
Slots are ordered small-to-large so attention starts before K/W' tiles
finish loading.

On-device compute uses fp16 matmul operands (fp32 PSUM accumulation),
fp32 softmax. The causal structure is exploited by giving each query-block
"slot" a fixed key extent (structure shared by all cores so one SPMD
program serves all 8); the actual mask enters as an additive bias built
on the host, so non-causal masks fall back to full-extent slots.
"""
import sys
sys.path.insert(0, '/opt/trn_rl_repo')
import math
from contextlib import ExitStack

import numpy as np

import concourse.bass as bass  # noqa: F401  (registers engines)
import concourse.mybir as mybir
import concourse.tile as tile
from concourse import bacc
from concourse.masks import make_identity

F32 = mybir.dt.float32
F16 = mybir.dt.float16

N_CORES = 8
B, S, D = 4, 2048, 2048
P = 128
NBLK = S // P            # 16 query blocks per batch
SQ = S // 2              # 1024 query rows per core
DCH = D // P             # 16 feature chunks
HALF = D // 2            # rope half dim

CAUSAL_SLOT_CHUNKS = [16, 14, 12, 10, 8, 6, 4, 2]
BLOCKS_EVEN = [15, 13, 11, 9, 6, 4, 2, 0]
BLOCKS_ODD = [14, 12, 10, 8, 7, 5, 3, 1]
FULL_SLOT_CHUNKS = [16] * 8

REPLICA_GROUPS = [[0, 1], [2, 3], [4, 5], [6, 7]]
NEG = -30000.0


def _proj_to_eT(nc, tc, ctx, w_dram, x_sb, bias_sb, out_raw, psum_pool, wpool):
    """out_raw[e, s] = (x @ W.T + b).T for x given as xT in SBUF.

    w_dram: [D, E] (= W.T, host-transposed), x_sb: [128, DCH, SQ] f16,
    bias_sb: [128, ECH] f32, out_raw: [128, ECH, SQ] f32 SBUF.
    """
    ech = out_raw.shape[1]
    sgroups = x_sb.shape[2] // 512
    for e2 in range(ech // 2):
        wts = []
        for d in range(DCH):
            wt = wpool.tile([P, 256], F16, tag="w")
            nc.sync.dma_start(wt[:], w_dram[e2, d])
            wts.append(wt)
        for es in range(2):
            e = e2 * 2 + es
            for sg in range(sgroups):
                ps = psum_pool.tile([P, 512], F32, tag="mm512")
                for d in range(DCH):
                    nc.tensor.matmul(
                        ps[:], wts[d][:, es * P:(es + 1) * P],
                        x_sb[:, d, sg * 512:(sg + 1) * 512],
                        start=(d == 0), stop=(d == DCH - 1))
                nc.vector.tensor_scalar_add(
                    out_raw[:, e, sg * 512:(sg + 1) * 512], ps[:], bias_sb[:, e:e + 1])


def _rope_to_stage(nc, raw, cos_sb, sin_sb, stage_dram, tmp_pool):
    """raw: [128, DCH, SQ] f32 (feature-major), cos/sin: [128, HALF//P, SQ] f32.
    Writes rope(raw) as f16 to stage_dram [D, SQ]."""
    hch = HALF // P  # 8
    sq = raw.shape[2]
    for c in range(hch):
        t1 = tmp_pool.tile([P, sq], F32, tag="rt1")
        t2 = tmp_pool.tile([P, sq], F32, tag="rt2")
        lo = tmp_pool.tile([P, sq], F16, tag="rlo")
        nc.vector.tensor_mul(t1[:], raw[:, c], cos_sb[:, c])
        nc.vector.tensor_mul(t2[:], raw[:, c + hch], sin_sb[:, c])
        nc.vector.tensor_sub(lo[:], t1[:], t2[:])
        nc.sync.dma_start(stage_dram[c * P:(c + 1) * P, :], lo[:])
        t3 = tmp_pool.tile([P, sq], F32, tag="rt1")
        t4 = tmp_pool.tile([P, sq], F32, tag="rt2")
        hi = tmp_pool.tile([P, sq], F16, tag="rlo")
        nc.vector.tensor_mul(t3[:], raw[:, c], sin_sb[:, c])
        nc.vector.tensor_mul(t4[:], raw[:, c + hch], cos_sb[:, c])
        nc.vector.tensor_add(hi[:], t3[:], t4[:])
        nc.sync.dma_start(stage_dram[(c + hch) * P:(c + hch + 1) * P, :], hi[:])


def build_program(slot_chunks, repeat=1, phases="all"):
    slot_chunks = list(slot_chunks)
    total_cols = sum(slot_chunks) * P
    nc = bacc.Bacc("TRN2", target_bir_lowering=False, debug=False, num_devices=N_CORES)

    xq_t = nc.dram_tensor("xq_t", [D, SQ], F16, kind="ExternalInput")
    xkv_t = nc.dram_tensor("xkv_t", [D, SQ], F16, kind="ExternalInput")
    wq_t = nc.dram_tensor("wq_tl", [D // 256, DCH, P, 256], F16, kind="ExternalInput")
    wk_t = nc.dram_tensor("wk_tl", [D // 256, DCH, P, 256], F16, kind="ExternalInput")
    wf_t = nc.dram_tensor("wf_t", [D, D], F16, kind="ExternalInput")
    bq_d = nc.dram_tensor("bq", [D], F32, kind="ExternalInput")
    bk_d = nc.dram_tensor("bk", [D], F32, kind="ExternalInput")
    bfo_d = nc.dram_tensor("bfo16", [D], F16, kind="ExternalInput")
    cosq_d = nc.dram_tensor("cosq", [HALF, SQ], F16, kind="ExternalInput")
    sinq_d = nc.dram_tensor("sinq", [HALF, SQ], F16, kind="ExternalInput")
    cosk_d = nc.dram_tensor("cosk", [HALF, SQ], F16, kind="ExternalInput")
    sink_d = nc.dram_tensor("sink", [HALF, SQ], F16, kind="ExternalInput")
    mbias_d = nc.dram_tensor("mbias", [P, total_cols], F16, kind="ExternalInput")
    out_d = nc.dram_tensor("out", [SQ, D], F32, kind="ExternalOutput")

    with tile.TileContext(nc) as tc, ExitStack() as ctx:
        dram = ctx.enter_context(tc.tile_pool(name="dram", bufs=1, space="DRAM"))
        const = ctx.enter_context(tc.tile_pool(name="const", bufs=1))
        psum_pool = ctx.enter_context(tc.tile_pool(name="psum", bufs=6, space="PSUM"))
        psum_t = ctx.enter_context(tc.tile_pool(name="psumT", bufs=2, space="PSUM"))

        ident = const.tile([P, P], F16)
        make_identity(nc, ident[:])
        ones1 = const.tile([1, P], F16)
        nc.vector.memset(ones1[:], 1.0)
        bq_sb = const.tile([P, DCH], F32)
        nc.sync.dma_start(bq_sb[:], bq_d.ap().rearrange("(o p) -> p o", p=P))
        bk_sb = const.tile([P, DCH], F32)
        nc.sync.dma_start(bk_sb[:], bk_d.ap().rearrange("(o p) -> p o", p=P))

        for _rep in range(repeat):
          kstage = dram.tile([D, SQ], F16)
          wstage_a = dram.tile([SQ // 2, D], F16)
          wstage_b = dram.tile([SQ // 2, D], F16)
          qstage = dram.tile([D, SQ], F16)
          kgather = dram.tile([2, D, SQ], F16)
          wgather_a = dram.tile([2, SQ // 2, D], F16)
          wgather_b = dram.tile([2, SQ // 2, D], F16)
          if _rep == repeat - 1:
              out_ap = out_d.ap()
          else:
              out_scratch = dram.tile([SQ, D], F32, name=f"out_scratch_{_rep}")
              out_ap = out_scratch[:]
          if phases == "none":
              ot = const.tile([1, 512], F32, name=f"dummy_out0_{_rep}")
              nc.vector.memset(ot[:], 1.0)
              nc.sync.dma_start(out_ap[0:1, 0:512], ot[:])
              continue

          # ---------------- P1: projections + allgather ----------------
          if True:
            with tc.tile_pool(name="kvx", bufs=1) as kvx:
              bfo_row = kvx.tile([1, D], F16, name=f"bfo_row_{_rep}")
              nc.sync.dma_start(bfo_row[:], bfo_d.ap()[None, :])
              xkv_sb = kvx.tile([P, DCH, SQ], F16)
              xkv_r = xkv_t.ap().rearrange("(do di) s -> di do s", di=P)
              for d in range(DCH):
                  nc.sync.dma_start(xkv_sb[:, d], xkv_r[:, d])

              # K projection + rope -> kstage -> allgather
              with tc.tile_pool(name="kp", bufs=1) as kp, \
                   tc.tile_pool(name="kw", bufs=24) as kw, \
                   tc.tile_pool(name="kt", bufs=1) as ktmp:
                  cosk_sb = kp.tile([P, HALF // P, SQ], F16)
                  nc.sync.dma_start(cosk_sb[:], cosk_d.ap().rearrange("(ho hi) s -> hi ho s", hi=P))
                  sink_sb = kp.tile([P, HALF // P, SQ], F16)
                  nc.sync.dma_start(sink_sb[:], sink_d.ap().rearrange("(ho hi) s -> hi ho s", hi=P))
                  kraw = kp.tile([P, DCH, SQ], F32)
                  _proj_to_eT(nc, tc, ctx, wk_t.ap(), xkv_sb, bk_sb, kraw, psum_pool, kw)
                  _rope_to_stage(nc, kraw, cosk_sb, sink_sb, kstage, ktmp)
              nc.gpsimd.collective_compute(
                  "AllGather", mybir.AluOpType.bypass, replica_groups=REPLICA_GROUPS,
                  ins=[kstage[:]], outs=[kgather[:]])

              # W' = x @ Wf.T + (Wo bv + bo) projection -> wstage -> allgather
              with tc.tile_pool(name="vw", bufs=1) as vw, \
                   tc.tile_pool(name="vs", bufs=3) as vstg:
                  wf_tiles = []
                  for d in range(DCH):
                      wt = vw.tile([P, D], F16, name=f"wf_{d}")
                      nc.sync.dma_start(wt[:], wf_t.ap()[d * P:(d + 1) * P, :])
                      wf_tiles.append(wt)
                  for sc in range(SQ // P):
                      whalf, wrow = (wstage_a, sc) if sc < 4 else (wstage_b, sc - 4)
                      for eg in range(4):
                          ps = psum_pool.tile([P, 512], F32, tag="mm512")
                          for d in range(DCH):
                              nc.tensor.matmul(
                                  ps[:], xkv_sb[:, d, sc * P:(sc + 1) * P],
                                  wf_tiles[d][:, eg * 512:(eg + 1) * 512],
                                  start=(d == 0), stop=False)
                          nc.tensor.matmul(
                              ps[:], ones1[:], bfo_row[:, eg * 512:(eg + 1) * 512],
                              start=False, stop=True)
                          st = vstg.tile([P, 512], F16, tag="vst")
                          nc.vector.tensor_copy(st[:], ps[:])
                          nc.sync.dma_start(
                              whalf[wrow * P:(wrow + 1) * P, eg * 512:(eg + 1) * 512], st[:])
                      if sc == 3:
                          nc.gpsimd.collective_compute(
                              "AllGather", mybir.AluOpType.bypass,
                              replica_groups=REPLICA_GROUPS,
                              ins=[wstage_a[:]], outs=[wgather_a[:]])
              nc.gpsimd.collective_compute(
                  "AllGather", mybir.AluOpType.bypass, replica_groups=REPLICA_GROUPS,
                  ins=[wstage_b[:]], outs=[wgather_b[:]])

            # Q projection + rope -> qstage (overlaps the allgathers)
            with tc.tile_pool(name="qp", bufs=1) as qp, \
                 tc.tile_pool(name="qw", bufs=24) as qw, \
                 tc.tile_pool(name="qt", bufs=1) as qtmp:
                xq_sb = qp.tile([P, DCH, SQ], F16)
                xq_r = xq_t.ap().rearrange("(do di) s -> di do s", di=P)
                for d in range(DCH):
                    nc.sync.dma_start(xq_sb[:, d], xq_r[:, d])
                cosq_sb = qp.tile([P, HALF // P, SQ], F16)
                nc.sync.dma_start(cosq_sb[:], cosq_d.ap().rearrange("(ho hi) s -> hi ho s", hi=P))
                sinq_sb = qp.tile([P, HALF // P, SQ], F16)
                nc.sync.dma_start(sinq_sb[:], sinq_d.ap().rearrange("(ho hi) s -> hi ho s", hi=P))
                qraw = qp.tile([P, DCH, SQ], F32)
                _proj_to_eT(nc, tc, ctx, wq_t.ap(), xq_sb, bq_sb, qraw, psum_pool, qw)
                _rope_to_stage(nc, qraw, cosq_sb, sinq_sb, qstage, qtmp)

          if phases == "p1":
              ot = const.tile([1, 512], F32, name=f"dummy_out_{_rep}")
              nc.vector.memset(ot[:], 1.0)
              nc.sync.dma_start(out_ap[0:1, 0:512], ot[:])
              continue

          # -------- P2: attention (+ folded output), block-pipelined --------
          with tc.tile_pool(name="attr", bufs=1) as attr, \
               tc.tile_pool(name="slot2", bufs=2) as sl2, \
               tc.tile_pool(name="slots", bufs=2) as sls, \
               tc.tile_pool(name="slot1", bufs=2) as sl1, \
               tc.tile_pool(name="ost", bufs=3) as ost:
              kT_sb = attr.tile([P, DCH, S], F16)
              nc.sync.dma_start(
                  kT_sb[:, :, 0:SQ],
                  kgather[0].rearrange("(do di) s -> di do s", di=P))
              nc.sync.dma_start(
                  kT_sb[:, :, SQ:S],
                  kgather[1].rearrange("(do di) s -> di do s", di=P))
              w_sb = attr.tile([P, NBLK, D], F16)
              nc.gpsimd.dma_start(
                  w_sb[:, 0:4, :],
                  wgather_a[0].rearrange("(co ci) e -> ci co e", ci=P))
              nc.gpsimd.dma_start(
                  w_sb[:, 4:8, :],
                  wgather_b[0].rearrange("(co ci) e -> ci co e", ci=P))
              nc.gpsimd.dma_start(
                  w_sb[:, 8:12, :],
                  wgather_a[1].rearrange("(co ci) e -> ci co e", ci=P))
              nc.gpsimd.dma_start(
                  w_sb[:, 12:16, :],
                  wgather_b[1].rearrange("(co ci) e -> ci co e", ci=P))

              def emit_tail(st):
                  kc_, j_, pexp_, linv_ = st
                  pT = sl1.tile([P, NBLK, P], F16, tag="pT")
                  for c in range(kc_):
                      pst = psum_t.tile([P, P], F16, tag="pst")
                      nc.tensor.transpose(pst[:], pexp_[:, c * P:(c + 1) * P], ident[:])
                      nc.vector.tensor_copy(pT[:, c, :], pst[:])
                  for eg in range(4):
                      pc = psum_pool.tile([P, 512], F32, tag="mm512")
                      for c in range(kc_):
                          nc.tensor.matmul(
                              pc[:], pT[:, c, :], w_sb[:, c, eg * 512:(eg + 1) * 512],
                              start=(c == 0), stop=(c == kc_ - 1))
                      ot = ost.tile([P, 512], F32, tag="ot")
                      nc.vector.tensor_scalar_mul(ot[:], pc[:], linv_[:])
                      nc.sync.dma_start(
                          out_ap[j_ * P:(j_ + 1) * P, eg * 512:(eg + 1) * 512], ot[:])

              off = 0
              prev = None
              for j, kc in enumerate(slot_chunks):
                  kw_cols = kc * P
                  qt = sl2.tile([P, DCH, P], F16, tag="qt")
                  nc.sync.dma_start(
                      qt[:], qstage[:, j * P:(j + 1) * P]
                      .rearrange("(do di) s -> di do s", di=P))
                  mb = sl2.tile([P, 2048], F16, tag="mb")
                  nc.sync.dma_start(mb[:, :kw_cols], mbias_d.ap()[:, off:off + kw_cols])
                  s_sb = sl1.tile([P, 2048], F32, tag="s")
                  for kg in range((kc + 3) // 4):
                      width = min(512, kw_cols - kg * 512)
                      ps = psum_pool.tile([P, 512], F32, tag="mm512")
                      for d in range(DCH):
                          nc.tensor.matmul(
                              ps[:, :width], qt[:, d, :],
                              kT_sb[:, d, kg * 512:kg * 512 + width],
                              start=(d == 0), stop=(d == DCH - 1))
                      nc.vector.tensor_add(
                          s_sb[:, kg * 512:kg * 512 + width],
                          ps[:, :width], mb[:, kg * 512:kg * 512 + width])
                  nm = sls.tile([P, 1], F32, tag="nm")
                  nc.vector.reduce_max(
                      nm[:], s_sb[:, :kw_cols], axis=mybir.AxisListType.X, negate=True)
                  lsum = sls.tile([P, 1], F32, tag="lsum")
                  pexp = sl1.tile([P, 2048], F16, tag="pexp")
                  nc.scalar.activation(
                      pexp[:, :kw_cols], s_sb[:, :kw_cols],
                      mybir.ActivationFunctionType.Exp,
                      bias=nm[:], accum_out=lsum[:])
                  linv = sls.tile([P, 1], F32, tag="linv")
                  nc.vector.reciprocal(linv[:], lsum[:])
                  if prev is not None:
                      emit_tail(prev)
                  prev = (kc, j, pexp, linv)
                  off += kw_cols
              emit_tail(prev)

    nc.compile()
    return nc


# ---------------- host side ----------------

_CACHE = {}


def _get_runner(slot_key):
    if slot_key not in _CACHE:
        nc = build_program(list(slot_key))
        from concourse.bass_utils import run_bass_kernel_spmd  # noqa: F401
        _CACHE[slot_key] = nc
    return _CACHE[slot_key]


def _tile_w(W):
    wt = np.ascontiguousarray(W.T).astype(np.float16)          # [D, E]
    wt = wt.reshape(DCH, P, D // 256, 256)                     # [d_out, d_in, e2, 256]
    return np.ascontiguousarray(wt.transpose(2, 0, 1, 3))      # [e2, d_out, 128, 256]


def _host_inputs(x, mask, Wq, bq, Wk, bk, Wv, bv, Wo, bo, slot_chunks, causal):
    """Build the 8 per-core input dicts."""
    scale = 1.0 / math.sqrt(D)
    inv_freq = 1.0 / (10000.0 ** (np.arange(HALF, dtype=np.float64) / HALF))
    pos = np.arange(S, dtype=np.float64)
    ang = pos[:, None] * inv_freq[None, :]          # [S, HALF]
    cos_full = np.cos(ang).astype(np.float32)       # [S, HALF]
    sin_full = np.sin(ang).astype(np.float32)

    Wf = np.asarray(Wo, np.float32) @ np.asarray(Wv, np.float32)
    bfo = np.asarray(Wo, np.float32) @ np.asarray(bv, np.float32) \
        + np.asarray(bo, np.float32)

    shared = {
        "wq_tl": _tile_w(Wq),
        "wk_tl": _tile_w(Wk),
        "wf_t": np.ascontiguousarray(Wf.T).astype(np.float16),
        "bq": np.asarray(bq, np.float32), "bk": np.asarray(bk, np.float32),
        "bfo16": bfo.astype(np.float16),
    }

    in_maps = []
    meta = []
    for c in range(N_CORES):
        b, h = c // 2, c % 2
        blocks = (BLOCKS_EVEN if h == 0 else BLOCKS_ODD)
        qrows = np.concatenate([np.arange(blk * P, (blk + 1) * P) for blk in blocks])
        kvrows = np.arange(h * SQ, (h + 1) * SQ)
        m = dict(shared)
        m["xq_t"] = np.ascontiguousarray(x[b][qrows].T).astype(np.float16)
        m["xkv_t"] = np.ascontiguousarray(x[b][kvrows].T).astype(np.float16)
        m["cosq"] = np.ascontiguousarray(cos_full[qrows].T * scale).astype(np.float16)
        m["sinq"] = np.ascontiguousarray(sin_full[qrows].T * scale).astype(np.float16)
        m["cosk"] = np.ascontiguousarray(cos_full[kvrows].T).astype(np.float16)
        m["sink"] = np.ascontiguousarray(sin_full[kvrows].T).astype(np.float16)
        mb_parts = []
        for j, kc in enumerate(slot_chunks):
            blk = blocks[j]
            rows = slice(blk * P, (blk + 1) * P)
            mm = mask[b, rows, :kc * P]
            mb_parts.append(np.where(mm == 0, np.float16(NEG), np.float16(0.0)))
        m["mbias"] = np.concatenate(mb_parts, axis=1).astype(np.float16)
        in_maps.append(m)
        meta.append((b, blocks))
    return in_maps, meta


def kernel(**inputs):
    x = np.asarray(inputs["x"], np.float32)
    mask = np.asarray(inputs["mask"])
    args = {k: np.asarray(inputs[k]) for k in
            ["Wq", "bq", "Wk", "bk", "Wv", "bv", "Wo", "bo"]}

    tril = np.tril(np.ones((S, S), dtype=mask.dtype))
    causal = all(np.array_equal(mask[b], tril) for b in range(B))
    slot_chunks = CAUSAL_SLOT_CHUNKS if causal else FULL_SLOT_CHUNKS

    in_maps, meta = _host_inputs(
        x, mask, args["Wq"], args["bq"], args["Wk"], args["bk"],
        args["Wv"], args["bv"], args["Wo"], args["bo"], slot_chunks, causal)

    nc = _get_runner(tuple(slot_chunks))
    from concourse.bass_utils import run_bass_kernel_spmd
    res = run_bass_kernel_spmd(nc, in_maps, list(range(N_CORES)))

    out = np.empty((B, S, D), np.float32)
    for c in range(N_CORES):
        b, blocks = meta[c]
        oc = res.results[c]["out"]
        for j, blk in enumerate(blocks):
            out[b, blk * P:(blk + 1) * P, :] = oc[j * P:(j + 1) * P, :]
    return out
